# revision 15
# baseline (speedup 1.0000x reference)
"""Trainium2 Bass kernel for nn_Gate_Net (Toeplitz + hard-sigmoid prob + cumprod gate).

Reference computation (per document row of 1024 scores):
  s = doc[1:-1]                      # n = 1022
  score_hat[r, j] = s[j-1-r] if j-1-r >= 0 else 0      # [1021, 1022]
  p[r, j] = clamp(10*(score_hat - s[j]) + 1, 0, 1)      # hard branch, res=0.1
  fwd = cumprod(p, axis=0); bwd = same with s reversed
  out = stack([fwd, bwd]) per doc -> full [32, 2, 1021, 1022] f32

Device algorithm (per doc-direction, column-major, j on partitions,
128-column block jb; partition p holds column j = jb*128 + 127 - p):
  Rows [0, 128):  q = min(B_shear + c_j, 1) (DVE tensor_scalar), cumprod via
    tensor_tensor_scan with (op0=mult, op1=max vs 0) - the max applies the
    lower clamp (state >= 0 makes max(q*state, 0) == clamp(q,0,1)*state).
    The bf16 scan *output* is load-bearing: f32-out mult/max scans run ~7x
    slower on DVE.
  Rows [128, 1021): closed form state_127 * clamp(c_j,0,1)^(r-127).
    Valid whenever each column either (a) has head length <= 128, or (b) its
    running product hits EXACT zero within the first 128 rows (factors are
    <= 0 with prob ~0.46/row, so P(violation) ~ 0.54^128 ~ 4e-35 per
    column).  kernel() VERIFIES this exactly on the host (same f32 ops)
    and recomputes any violating column exactly - so the cap is an exact
    algorithm with a never-taken slow path, not an approximation.
    Evaluated as ScalarE Exp(iota * lc + bias) with per-partition AP
    scale/bias (lc = log(clamp(c,0,1)+1e-30) from the host; one ACT table,
    no Ln), then multiplied by the per-column scan state on DVE.
  All 8 blocks land packed in one [128, 8*1021] bf16 SBUF tile, stored in
  2-block chunks (4084B partition lines), COLUMN-major; the host gather
  undoes the (block, reversed-partition) permutation and upcasts to f32.

Sharding: pure data parallel, 8 doc-dirs per core (4 docs x fwd/bwd).
"""
import numpy as np

import concourse.bass as bass
import concourse.bacc as bacc
import concourse.tile as tile
from concourse import mybir
from concourse import bass_utils

P = 128
N = 1022          # columns j per doc-dir
ROWS = N - 1      # 1021 output rows
NB = 8            # column blocks (last has 126 valid columns)
ARRW = 1152       # padded diag-source array width
BW = 1024         # sheared tile width
L = 128           # scan cap (rows scanned per block)
TAIL = ROWS - L   # 893 geometric rows per block
LOG_TINY = np.float32(1e-30)
BF16 = mybir.dt.bfloat16
NPBF16 = mybir.dt.np(BF16)

_NC_CACHE: dict = {}


def build_nc(n_dd: int = 8):
    """Build the single-core Bass program processing n_dd doc-dirs."""
    nc = bacc.Bacc("TRN2", target_bir_lowering=False, debug=False, num_devices=8)
    arr = nc.dram_tensor("arr", [n_dd, ARRW], mybir.dt.float32, kind="ExternalInput")
    cc = nc.dram_tensor("cc", [n_dd, P, 24], mybir.dt.float32, kind="ExternalInput")
    out = nc.dram_tensor("out", [n_dd, P, NB * ROWS], BF16, kind="ExternalOutput")

    add_op = mybir.AluOpType.add
    min_op = mybir.AluOpType.min
    mult_op = mybir.AluOpType.mult
    max_op = mybir.AluOpType.max
    exp_fn = mybir.ActivationFunctionType.Exp

    with tile.TileContext(nc) as tc:
        with (
            tc.tile_pool(name="consts", bufs=1) as consts,
            tc.tile_pool(name="bsrc", bufs=3) as bsrc_pool,
            tc.tile_pool(name="qpool", bufs=3) as qpool,
            tc.tile_pool(name="expool", bufs=4) as expool,
            tc.tile_pool(name="rpool", bufs=2) as rpool,
            tc.tile_pool(name="cpool", bufs=3) as cpool,
            tc.tile_pool(name="spool", bufs=4) as spool,
        ):
            zeros = consts.tile([P, L], mybir.dt.float32)
            nc.vector.memset(zeros[:], 0.0)
            iota = consts.tile([P, TAIL], mybir.dt.float32)
            nc.gpsimd.iota(
                iota[:], pattern=[[1, TAIL]], base=L, channel_multiplier=0,
                allow_small_or_imprecise_dtypes=True,
            )

            for dd in range(n_dd):
                B = bsrc_pool.tile([P, BW], mybir.dt.float32, tag="B")
                # jb=0 only needs B[:, 896:1024]; land that slice first.
                nc.sync.dma_start(
                    out=B[:, 896:BW],
                    in_=bass.AP(
                        tensor=arr, offset=dd * ARRW + 896, ap=[[1, P], [1, BW - 896]]
                    ),
                )
                nc.sync.dma_start(
                    out=B[:, 0:896],
                    in_=bass.AP(
                        tensor=arr, offset=dd * ARRW, ap=[[1, P], [1, 896]]
                    ),
                )

                csb = cpool.tile([P, 24], mybir.dt.float32, tag="csb")
                nc.sync.dma_start(out=csb[:], in_=cc[dd, :, :])

                R = rpool.tile([P, NB * ROWS], BF16, tag="R", name="R")
                for jb in range(NB):
                    y = 896 - jb * 128
                    o = jb * ROWS
                    # geometric tail values exp(r*lc + (1-L)*lc), r = L..1020;
                    # depends only on host constants, so ScalarE runs ahead.
                    Ex = expool.tile([P, TAIL], BF16, tag="Ex")
                    nc.scalar.activation(
                        out=Ex[:], in_=iota[:],
                        func=exp_fn,
                        bias=csb[:, 16 + jb:17 + jb],
                        scale=csb[:, 8 + jb:9 + jb],
                    )
                    # head factors: q = min(B_slice + c_j, 1); lower clamp
                    # happens inside the scan (op1 = max vs 0).
                    Q = qpool.tile([P, L], mybir.dt.float32, tag="Q", name="Q")
                    nc.vector.tensor_scalar(
                        out=Q[:],
                        in0=B[:, y:y + L],
                        scalar1=csb[:, jb:jb + 1],
                        scalar2=1.0,
                        op0=add_op,
                        op1=min_op,
                    )
                    nc.vector.tensor_tensor_scan(
                        out=R[:, o:o + L],
                        data0=Q[:],
                        data1=zeros[:],
                        initial=1.0,
                        op0=mult_op,
                        op1=max_op,
                    )
                    st = spool.tile([P, 1], mybir.dt.float32, tag="st")
                    nc.vector.tensor_copy(st[:], R[:, o + L - 1:o + L])
                    nc.vector.tensor_scalar(
                        out=R[:, o + L:o + ROWS],
                        in0=Ex[:],
                        scalar1=st[:],
                        scalar2=None,
                        op0=mult_op,
                    )
                    if jb % 2 == 1:
                        # store via the idle SWDGE (gpsimd) queue: the sync
                        # queue stays pure input prefetch, so a store waiting
                        # on compute never delays the next doc-dir's loads.
                        lo, hi = (jb - 1) * ROWS, (jb + 1) * ROWS
                        nc.gpsimd.dma_start(out=out[dd, :, lo:hi], in_=R[:, lo:hi])
    nc.compile()
    return nc


def get_nc(n_dd: int = 8):
    if n_dd not in _NC_CACHE:
        _NC_CACHE[n_dd] = build_nc(n_dd)
    return _NC_CACHE[n_dd]


def make_core_inputs(docs_core: np.ndarray) -> dict:
    """docs_core: [n_docs, 1024] f32 -> in_map with arr/cc."""
    n_docs = docs_core.shape[0]
    n_dd = n_docs * 2
    arr = np.zeros((n_dd, ARRW), np.float32)
    cc = np.zeros((n_dd, P, 24), np.float32)
    for dl in range(n_docs):
        s = docs_core[dl, 1:-1].astype(np.float32)  # 1022
        for t in range(2):
            v = s if t == 0 else s[::-1]
            dd = dl * 2 + t
            v10 = (np.float32(10.0) * v).astype(np.float32)
            arr[dd, 1:1 + N] = v10[::-1]
            cvals = (np.float32(1.0) - v10).astype(np.float32)
            # partition p holds column j = jb*128 + (127 - p)
            for jb in range(NB):
                seg = cvals[jb * 128: jb * 128 + 128]
                cseg = np.zeros(P, np.float32)
                cseg[P - len(seg):] = seg[::-1]
                cc[dd, :, jb] = cseg
                lc = np.log(np.clip(cseg, 0.0, 1.0) + LOG_TINY).astype(np.float32)
                cc[dd, :, 8 + jb] = lc
                cc[dd, :, 16 + jb] = (np.float32(1 - L) * lc).astype(np.float32)
    return {"arr": arr, "cc": cc}


def make_in_maps(score: np.ndarray, score_idx: np.ndarray):
    """Helper for the test harness: full inputs -> per-core in_maps."""
    score = np.asarray(score, dtype=np.float32)
    docs = score[np.asarray(score_idx)]
    n_cores = 8
    dpc = docs.shape[0] // n_cores
    in_maps = [make_core_inputs(docs[c * dpc:(c + 1) * dpc]) for c in range(n_cores)]
    return in_maps, None


def _fix_violations(full, docs, in_maps, n_cores, docs_per_core):
    """Exact scan-cap check: a column may extend past row L-1 without an
    exact zero only with probability ~0.54^128; verify with the same f32
    arithmetic the device uses and recompute any such column exactly."""
    from numpy.lib.stride_tricks import as_strided

    for c in range(n_cores):
        arr, cc = in_maps[c]["arr"], in_maps[c]["cc"]
        for dd in range(docs_per_core * 2):
            a = arr[dd]
            for jb in range(NB):
                y = 896 - jb * 128
                win = as_strided(
                    a[y:], shape=(P, L),
                    strides=(a.strides[0], a.strides[0]),
                )
                q = np.minimum(win + cc[dd, :, jb:jb + 1], np.float32(1.0))
                no_zero = ~(q <= 0).any(axis=1)
                ps = np.nonzero(no_zero)[0]
                for p in ps:
                    j = jb * 128 + 127 - p
                    if j <= L or j >= N:
                        continue  # head fits in the scan / padding column
                    dl, t = dd // 2, dd % 2
                    doc = c * docs_per_core + dl
                    s = docs[doc, 1:-1].astype(np.float32)
                    v = s if t == 0 else s[::-1]
                    v10 = (np.float32(10.0) * v).astype(np.float32)
                    cj = np.float32(1.0) - v10[j]
                    f = np.full(ROWS, cj, np.float32)
                    f[:j] = v10[j - 1::-1] + cj
                    f = np.clip(np.minimum(f, np.float32(1.0)), 0.0, None)
                    full[doc, t, :, j] = np.cumprod(f, dtype=np.float32)
    return full


def kernel(score: np.ndarray, score_idx: np.ndarray) -> np.ndarray:
    score = np.asarray(score, dtype=np.float32)
    score_idx = np.asarray(score_idx)
    docs = score[score_idx]  # [B, L] gather
    Bn, Ln = docs.shape      # 32, 1024
    n_cores = 8
    docs_per_core = Bn // n_cores  # 4

    in_maps = [
        make_core_inputs(docs[c * docs_per_core:(c + 1) * docs_per_core])
        for c in range(n_cores)
    ]
    nc = get_nc(docs_per_core * 2)
    res = bass_utils.run_bass_kernel_spmd(nc, in_maps, core_ids=list(range(n_cores)))
    full = np.empty((Bn, 2, ROWS, N), np.float32)
    for c in range(n_cores):
        o = np.asarray(res.results[c]["out"])  # [n_dd, P, NB*ROWS] bf16
        o32 = o.astype(np.float32).reshape(docs_per_core * 2, P, NB, ROWS)
        # device R[p, r] holds out[r, j] for j = jb*128 + 127 - p:
        # reorder to [dd, jb, p, r], reverse partitions so n = 127 - p is
        # the in-block column, then [dd, jb, n, r] -> [dd, r, jb*128 + n].
        o32 = np.transpose(o32, (0, 2, 1, 3))[:, :, ::-1, :]
        o32 = np.transpose(o32, (0, 3, 1, 2)).reshape(
            docs_per_core * 2, ROWS, NB * P
        )[:, :, :N]
        for dl in range(docs_per_core):
            for t in range(2):
                full[c * docs_per_core + dl, t] = o32[dl * 2 + t]
    return _fix_violations(full, docs, in_maps, n_cores, docs_per_core)


# revision 17
# speedup vs baseline: 1.3053x; 1.3053x over previous
"""Trainium2 Bass kernel for nn_Gate_Net (Toeplitz + hard-sigmoid prob + cumprod gate).

Reference computation (per document row of 1024 scores):
  s = doc[1:-1]                      # n = 1022
  score_hat[r, j] = s[j-1-r] if j-1-r >= 0 else 0      # [1021, 1022]
  p[r, j] = clamp(10*(score_hat - s[j]) + 1, 0, 1)      # hard branch, res=0.1
  fwd = cumprod(p, axis=0); bwd = same with s reversed
  out = stack([fwd, bwd]) per doc -> full [32, 2, 1021, 1022] f32

Device algorithm (per doc-direction, column-major, j on partitions,
128-column block jb; partition p holds column j = jb*128 + 127 - p):
  Rows [0, 128):  q = min(B_shear + c_j, 1) (DVE tensor_scalar), cumprod via
    tensor_tensor_scan with (op0=mult, op1=max vs 0) - the max applies the
    lower clamp (state >= 0 makes max(q*state, 0) == clamp(q,0,1)*state).
    The bf16 scan *output* is load-bearing: f32-out mult/max scans run ~7x
    slower on DVE.
  Rows [128, 1021): closed form state_127 * clamp(c_j,0,1)^(r-127).
    Valid whenever each column either (a) has head length <= 128, or (b) its
    running product hits EXACT zero within the first 128 rows (factors are
    <= 0 with prob ~0.46/row, so P(violation) ~ 0.54^128 ~ 4e-35 per
    column).  kernel() VERIFIES this exactly on the host (same f32 ops)
    and recomputes any violating column exactly - so the cap is an exact
    algorithm with a never-taken slow path, not an approximation.
    Evaluated as ScalarE Exp(iota * lc + bias) with per-partition AP
    scale/bias (lc = log(clamp(c,0,1)+1e-30) from the host; one ACT table,
    no Ln), then multiplied by the per-column scan state on DVE.
  All 8 blocks land packed in one [128, 8*1021] bf16 SBUF tile, stored in
  2-block chunks (4084B partition lines), COLUMN-major; the host gather
  undoes the (block, reversed-partition) permutation and upcasts to f32.

Sharding: pure data parallel, 8 doc-dirs per core (4 docs x fwd/bwd).
"""
import numpy as np

import concourse.bass as bass
import concourse.bacc as bacc
import concourse.tile as tile
from concourse import mybir
from concourse import bass_utils

P = 128
N = 1022          # columns j per doc-dir
ROWS = N - 1      # 1021 output rows
NB = 8            # column blocks (last has 126 valid columns)
ARRW = 1152       # padded diag-source array width
BW = 1024         # sheared tile width
L = 128           # scan cap (rows scanned per block)
TAIL = ROWS - L   # 893 geometric rows per block
LOG_TINY = np.float32(1e-30)
BF16 = mybir.dt.bfloat16
NPBF16 = mybir.dt.np(BF16)

_NC_CACHE: dict = {}


def build_nc(n_dd: int = 8):
    """Build the single-core Bass program processing n_dd doc-dirs."""
    nc = bacc.Bacc("TRN2", target_bir_lowering=False, debug=False, num_devices=8)
    arr = nc.dram_tensor("arr", [n_dd, ARRW], mybir.dt.float32, kind="ExternalInput")
    cc = nc.dram_tensor("cc", [n_dd, P, 24], mybir.dt.float32, kind="ExternalInput")
    out = nc.dram_tensor("out", [n_dd, P, NB * ROWS], BF16, kind="ExternalOutput")

    add_op = mybir.AluOpType.add
    min_op = mybir.AluOpType.min
    mult_op = mybir.AluOpType.mult
    max_op = mybir.AluOpType.max
    exp_fn = mybir.ActivationFunctionType.Exp

    with tile.TileContext(nc) as tc:
        with (
            tc.tile_pool(name="consts", bufs=1) as consts,
            tc.tile_pool(name="bsrc", bufs=3) as bsrc_pool,
            tc.tile_pool(name="qpool", bufs=3) as qpool,
            tc.tile_pool(name="expool", bufs=4) as expool,
            tc.tile_pool(name="rpool", bufs=2) as rpool,
            tc.tile_pool(name="cpool", bufs=3) as cpool,
            tc.tile_pool(name="spool", bufs=4) as spool,
        ):
            zeros = consts.tile([P, L], mybir.dt.float32)
            nc.vector.memset(zeros[:], 0.0)
            iota = consts.tile([P, TAIL], mybir.dt.float32)
            nc.gpsimd.iota(
                iota[:], pattern=[[1, TAIL]], base=L, channel_multiplier=0,
                allow_small_or_imprecise_dtypes=True,
            )

            def load_inputs(dd):
                B = bsrc_pool.tile([P, BW], mybir.dt.float32, tag="B")
                # jb=0 only needs B[:, 896:1024]; land that slice first.
                nc.sync.dma_start(
                    out=B[:, 896:BW],
                    in_=bass.AP(
                        tensor=arr, offset=dd * ARRW + 896, ap=[[1, P], [1, BW - 896]]
                    ),
                )
                nc.sync.dma_start(
                    out=B[:, 0:896],
                    in_=bass.AP(
                        tensor=arr, offset=dd * ARRW, ap=[[1, P], [1, 896]]
                    ),
                )
                csb = cpool.tile([P, 24], mybir.dt.float32, tag="csb")
                nc.sync.dma_start(out=csb[:], in_=cc[dd, :, :])
                return B, csb

            # issue input DMAs two doc-dirs ahead of the stores in program
            # order, so a store waiting on compute never delays prefetch in
            # the sync queue's FIFO.
            loaded = [load_inputs(dd) for dd in range(min(2, n_dd))]
            for dd in range(n_dd):
                B, csb = loaded[dd]
                if dd + 2 < n_dd:
                    loaded.append(load_inputs(dd + 2))

                R = rpool.tile([P, NB * ROWS], BF16, tag="R", name="R")
                for jb in range(NB):
                    y = 896 - jb * 128
                    o = jb * ROWS
                    # geometric tail values exp(r*lc + (1-L)*lc), r = L..1020;
                    # depends only on host constants, so ScalarE runs ahead.
                    Ex = expool.tile([P, TAIL], BF16, tag="Ex")
                    nc.scalar.activation(
                        out=Ex[:], in_=iota[:],
                        func=exp_fn,
                        bias=csb[:, 16 + jb:17 + jb],
                        scale=csb[:, 8 + jb:9 + jb],
                    )
                    # head factors: q = min(B_slice + c_j, 1); lower clamp
                    # happens inside the scan (op1 = max vs 0).
                    Q = qpool.tile([P, L], mybir.dt.float32, tag="Q", name="Q")
                    nc.vector.tensor_scalar(
                        out=Q[:],
                        in0=B[:, y:y + L],
                        scalar1=csb[:, jb:jb + 1],
                        scalar2=1.0,
                        op0=add_op,
                        op1=min_op,
                    )
                    nc.vector.tensor_tensor_scan(
                        out=R[:, o:o + L],
                        data0=Q[:],
                        data1=zeros[:],
                        initial=1.0,
                        op0=mult_op,
                        op1=max_op,
                    )
                    st = spool.tile([P, 1], mybir.dt.float32, tag="st")
                    nc.vector.tensor_copy(st[:], R[:, o + L - 1:o + L])
                    nc.vector.tensor_scalar(
                        out=R[:, o + L:o + ROWS],
                        in0=Ex[:],
                        scalar1=st[:],
                        scalar2=None,
                        op0=mult_op,
                    )
                    if jb % 2 == 1:
                        lo, hi = (jb - 1) * ROWS, (jb + 1) * ROWS
                        nc.sync.dma_start(out=out[dd, :, lo:hi], in_=R[:, lo:hi])
    nc.compile()
    return nc


def get_nc(n_dd: int = 8):
    if n_dd not in _NC_CACHE:
        _NC_CACHE[n_dd] = build_nc(n_dd)
    return _NC_CACHE[n_dd]


def make_core_inputs(docs_core: np.ndarray) -> dict:
    """docs_core: [n_docs, 1024] f32 -> in_map with arr/cc."""
    n_docs = docs_core.shape[0]
    n_dd = n_docs * 2
    arr = np.zeros((n_dd, ARRW), np.float32)
    cc = np.zeros((n_dd, P, 24), np.float32)
    for dl in range(n_docs):
        s = docs_core[dl, 1:-1].astype(np.float32)  # 1022
        for t in range(2):
            v = s if t == 0 else s[::-1]
            dd = dl * 2 + t
            v10 = (np.float32(10.0) * v).astype(np.float32)
            arr[dd, 1:1 + N] = v10[::-1]
            cvals = (np.float32(1.0) - v10).astype(np.float32)
            # partition p holds column j = jb*128 + (127 - p)
            for jb in range(NB):
                seg = cvals[jb * 128: jb * 128 + 128]
                cseg = np.zeros(P, np.float32)
                cseg[P - len(seg):] = seg[::-1]
                cc[dd, :, jb] = cseg
                lc = np.log(np.clip(cseg, 0.0, 1.0) + LOG_TINY).astype(np.float32)
                cc[dd, :, 8 + jb] = lc
                cc[dd, :, 16 + jb] = (np.float32(1 - L) * lc).astype(np.float32)
    return {"arr": arr, "cc": cc}


def make_in_maps(score: np.ndarray, score_idx: np.ndarray):
    """Helper for the test harness: full inputs -> per-core in_maps."""
    score = np.asarray(score, dtype=np.float32)
    docs = score[np.asarray(score_idx)]
    n_cores = 8
    dpc = docs.shape[0] // n_cores
    in_maps = [make_core_inputs(docs[c * dpc:(c + 1) * dpc]) for c in range(n_cores)]
    return in_maps, None


def _fix_violations(full, docs, in_maps, n_cores, docs_per_core):
    """Exact scan-cap check: a column may extend past row L-1 without an
    exact zero only with probability ~0.54^128; verify with the same f32
    arithmetic the device uses and recompute any such column exactly."""
    from numpy.lib.stride_tricks import as_strided

    for c in range(n_cores):
        arr, cc = in_maps[c]["arr"], in_maps[c]["cc"]
        for dd in range(docs_per_core * 2):
            a = arr[dd]
            for jb in range(NB):
                y = 896 - jb * 128
                win = as_strided(
                    a[y:], shape=(P, L),
                    strides=(a.strides[0], a.strides[0]),
                )
                q = np.minimum(win + cc[dd, :, jb:jb + 1], np.float32(1.0))
                no_zero = ~(q <= 0).any(axis=1)
                ps = np.nonzero(no_zero)[0]
                for p in ps:
                    j = jb * 128 + 127 - p
                    if j <= L or j >= N:
                        continue  # head fits in the scan / padding column
                    dl, t = dd // 2, dd % 2
                    doc = c * docs_per_core + dl
                    s = docs[doc, 1:-1].astype(np.float32)
                    v = s if t == 0 else s[::-1]
                    v10 = (np.float32(10.0) * v).astype(np.float32)
                    cj = np.float32(1.0) - v10[j]
                    f = np.full(ROWS, cj, np.float32)
                    f[:j] = v10[j - 1::-1] + cj
                    f = np.clip(np.minimum(f, np.float32(1.0)), 0.0, None)
                    full[doc, t, :, j] = np.cumprod(f, dtype=np.float32)
    return full


def kernel(score: np.ndarray, score_idx: np.ndarray) -> np.ndarray:
    score = np.asarray(score, dtype=np.float32)
    score_idx = np.asarray(score_idx)
    docs = score[score_idx]  # [B, L] gather
    Bn, Ln = docs.shape      # 32, 1024
    n_cores = 8
    docs_per_core = Bn // n_cores  # 4

    in_maps = [
        make_core_inputs(docs[c * docs_per_core:(c + 1) * docs_per_core])
        for c in range(n_cores)
    ]
    nc = get_nc(docs_per_core * 2)
    res = bass_utils.run_bass_kernel_spmd(nc, in_maps, core_ids=list(range(n_cores)))
    full = np.empty((Bn, 2, ROWS, N), np.float32)
    for c in range(n_cores):
        o = np.asarray(res.results[c]["out"])  # [n_dd, P, NB*ROWS] bf16
        o32 = o.astype(np.float32).reshape(docs_per_core * 2, P, NB, ROWS)
        # device R[p, r] holds out[r, j] for j = jb*128 + 127 - p:
        # reorder to [dd, jb, p, r], reverse partitions so n = 127 - p is
        # the in-block column, then [dd, jb, n, r] -> [dd, r, jb*128 + n].
        o32 = np.transpose(o32, (0, 2, 1, 3))[:, :, ::-1, :]
        o32 = np.transpose(o32, (0, 3, 1, 2)).reshape(
            docs_per_core * 2, ROWS, NB * P
        )[:, :, :N]
        for dl in range(docs_per_core):
            for t in range(2):
                full[c * docs_per_core + dl, t] = o32[dl * 2 + t]
    return _fix_violations(full, docs, in_maps, n_cores, docs_per_core)
